# revision 10
# baseline (speedup 1.0000x reference)
"""Trainium2 Bass kernel for causal multi-head attention with RoPE.

Problem: B=4, T=2048, D=1024, H=16 heads (HD=64), fp32.
  q,k,v = x@W* + b*;  RoPE(q,k);  causal softmax(q k^T / 8) @ v;  out @ Wo + bo.

Sharding: tensor-parallel over heads — each of the 8 cores owns 2 heads
(a 128-column slice of Wq/Wk/Wv and 128-row slice of Wo) and computes a
partial [B*T, D] output; the host sums the 8 partials and adds bo.

Device layout (per core, everything transpose-free):
  - host passes x pre-transposed as xT [128, 8, B*T] (D on partitions,
    8 K-chunks along free dim) pre-rounded to fp32r
  - qT,kT [128(=2 heads x 64d), T] per batch via matmul(lhsT=W chunk,
    rhs=xT chunk); RoPE applied in [d,t] layout with host cosT/sinT
  - scores TRANSPOSED: S^T[key,q] = matmul(lhsT=kT head, rhs=qT head);
    softmax runs over the partition dim with no max-subtraction (scores
    here are bounded ~|s|<3), sums taken by a ones-column appended to V
  - V in natural [key, d] layout via PE transpose of vT
  - O^T accumulates in PSUM via matmul(lhsT=V_aug, rhs=exp(S^T)); row 64
    is the softmax denominator; normalize via reciprocal + ones-broadcast
  - output projection: matmul(lhsT=O^T chunk, rhs=Wo rows) -> partial out

fp32r (fp32 with 11-bit mantissa, 4x PE throughput) is used for all
matmul operands; accumulation stays fp32 in PSUM.
"""
import numpy as np
import sys

sys.path.insert(0, "/opt/trn_rl_repo")

import concourse.bass as bass
import concourse.mybir as mybir
import concourse.tile as tile
from concourse import bacc
from concourse.bass_utils import run_bass_kernel_spmd
from concourse.masks import make_identity

B, T, D, H = 4, 2048, 1024, 16
HD = D // H            # 64
NT = B * T             # 8192 tokens
NCORES = 8
HPC = H // NCORES      # 2 heads per core
DPC = HPC * HD         # 128 d per core
P = 128
KD = D // P            # 8 contraction chunks for the projections
TQ = 512               # q-chunk width (psum free dim)
NTT = NT // TQ         # 16 token tiles
TPB = T // TQ          # 4 q-chunks per batch
KCB = T // P           # 16 key chunks per batch
SCALE = 1.0 / np.sqrt(HD)

F32 = mybir.dt.float32
F32R = mybir.dt.float32r
AF = mybir.ActivationFunctionType
MULT = mybir.AluOpType.mult

_cache = {}
_last_in_maps = None


def round_fp32r(a: np.ndarray) -> np.ndarray:
    """Round-to-nearest-even to fp32 with 11-bit mantissa (fp32r storage)."""
    b = np.ascontiguousarray(a, dtype=np.float32).view(np.uint32)
    r = (b + 0x7FF + ((b >> 12) & 1)) & np.uint32(0xFFFFF000)
    return r.view(np.float32)


def build_nc(causal: bool, repeat: int = 1):
    nc = bacc.Bacc("TRN2", target_bir_lowering=False, debug=False,
                   enable_asserts=True, num_devices=NCORES)
    xt = nc.dram_tensor("xt", [P, KD, NT], F32R, kind="ExternalInput")
    wq = nc.dram_tensor("wq", [P, KD, DPC], F32R, kind="ExternalInput")
    wk = nc.dram_tensor("wk", [P, KD, DPC], F32R, kind="ExternalInput")
    wv = nc.dram_tensor("wv", [P, KD, DPC], F32R, kind="ExternalInput")
    wo = nc.dram_tensor("wo", [DPC, D], F32R, kind="ExternalInput")
    bq = nc.dram_tensor("bq", [DPC, 1], F32, kind="ExternalInput")
    bk = nc.dram_tensor("bk", [DPC, 1], F32, kind="ExternalInput")
    bv = nc.dram_tensor("bv", [DPC, 1], F32, kind="ExternalInput")
    cosd = nc.dram_tensor("cosd", [DPC, T], F32, kind="ExternalInput")
    sind = nc.dram_tensor("sind", [DPC, T], F32, kind="ExternalInput")
    diag = nc.dram_tensor("diag", [P, 4, TQ], F32, kind="ExternalInput")
    out = nc.dram_tensor("out", [NT, D], F32, kind="ExternalOutput")

    with tile.TileContext(nc) as tc:
        with (
            tc.tile_pool(name="const", bufs=1) as const_pool,
            tc.tile_pool(name="xs", bufs=2) as x_pool,
            tc.tile_pool(name="qk", bufs=2) as qk_pool,
            tc.tile_pool(name="work", bufs=2) as work_pool,
            tc.tile_pool(name="expp", bufs=4) as exp_pool,
            tc.tile_pool(name="ot", bufs=2) as ot_pool,
            tc.tile_pool(name="outp", bufs=3) as out_pool,
            tc.tile_pool(name="mm512", bufs=2, space="PSUM") as psum_mm,
            tc.tile_pool(name="spsum", bufs=2, space="PSUM") as psum_s,
            tc.tile_pool(name="opsum", bufs=2, space="PSUM") as psum_o,
            tc.tile_pool(name="smallp", bufs=2, space="PSUM") as psum_small,
        ):
            # resident constants
            wq_sb = const_pool.tile([P, KD, DPC], F32R)
            wk_sb = const_pool.tile([P, KD, DPC], F32R)
            wv_sb = const_pool.tile([P, KD, DPC], F32R)
            wo_sb = const_pool.tile([DPC, D], F32R)
            bq_sb = const_pool.tile([DPC, 1], F32)
            bk_sb = const_pool.tile([DPC, 1], F32)
            bv_sb = const_pool.tile([DPC, 1], F32)
            cos_sb = const_pool.tile([DPC, T], F32)
            sin_sb = const_pool.tile([DPC, T], F32)
            ident = const_pool.tile([P, P], F32)
            ones_sb = const_pool.tile([1, HD], F32R)
            onesf_sb = const_pool.tile([1, HD], F32)
            nc.sync.dma_start(wq_sb[:], wq[:])
            nc.sync.dma_start(wk_sb[:], wk[:])
            nc.sync.dma_start(wv_sb[:], wv[:])
            nc.sync.dma_start(wo_sb[:], wo[:])
            nc.sync.dma_start(bq_sb[:], bq[:])
            nc.sync.dma_start(bk_sb[:], bk[:])
            nc.sync.dma_start(bv_sb[:], bv[:])
            nc.sync.dma_start(cos_sb[:], cosd[:])
            nc.sync.dma_start(sin_sb[:], sind[:])
            make_identity(nc, ident[:])
            nc.vector.memset(onesf_sb[:], 1.0)
            nc.vector.tensor_copy(ones_sb[:], onesf_sb[:])
            if causal:
                diag_sb = const_pool.tile([P, 4, TQ], F32)
                nc.sync.dma_start(diag_sb[:], diag[:])

            onescol_f = const_pool.tile([P, KCB, HPC, 1], F32)
            nc.vector.memset(onescol_f[:], 1.0)

            def rope(dst, src_psum, bias_sb, tt):
                """psum [128, TQ] -> dst slice with bias + RoPE, in [d,t] layout.
                sin_sb holds SIGNED sin (rows p%64<32 negated), so rotate-half
                is 4 plain partition-shifted copies (1-input ops may shift)."""
                S = bass.ds(tt * TQ, TQ)
                raw = work_pool.tile([P, TQ], F32, tag="rope_raw")
                nc.scalar.activation(raw[:], src_psum[:], AF.Identity, bias=bias_sb[:])
                rot = work_pool.tile([P, TQ], F32, tag="rope_rot")
                h2 = HD // 2
                for h in range(HPC):
                    o = h * HD
                    nc.vector.tensor_copy(rot[o:o + h2, :], raw[o + h2:o + HD, :])
                    nc.vector.tensor_copy(rot[o + h2:o + HD, :], raw[o:o + h2, :])
                nc.vector.tensor_tensor(rot[:], rot[:], sin_sb[:, S], MULT)
                nc.vector.tensor_tensor(dst[:, S], raw[:], cos_sb[:, S], MULT)
                nc.vector.tensor_add(dst[:, S], dst[:, S], rot[:])

            for rep in range(repeat):
              for b in range(B):
                # ---- projections for batch b: qT, kT, vA ----
                qT = qk_pool.tile([P, T], F32R, tag="qT")
                kT = qk_pool.tile([P, T], F32R, tag="kT")
                vA = qk_pool.tile([P, KCB, HPC, HD + 1], F32R, tag="vA")
                nc.vector.tensor_copy(vA[:, :, :, HD:HD + 1], onescol_f[:])
                for tt in range(TPB):
                    gt = b * TPB + tt  # global token tile
                    xt_sb = x_pool.tile([P, KD, TQ], F32R, tag="xt")
                    nc.sync.dma_start(xt_sb[:], xt[:, :, bass.ds(gt * TQ, TQ)])
                    for w_sb, b_sb, dstT in ((wq_sb, bq_sb, qT), (wk_sb, bk_sb, kT)):
                        mm = psum_mm.tile([P, TQ], F32, tag="mm512")
                        for kc in range(KD):
                            nc.tensor.matmul(mm[:], w_sb[:, kc, :], xt_sb[:, kc, :],
                                             start=(kc == 0), stop=(kc == KD - 1))
                        rope(dstT, mm, b_sb, tt)
                    # v: compute vT then PE-transpose into natural layout
                    mmv = psum_mm.tile([P, TQ], F32, tag="mm512")
                    for kc in range(KD):
                        nc.tensor.matmul(mmv[:], wv_sb[:, kc, :], xt_sb[:, kc, :],
                                         start=(kc == 0), stop=(kc == KD - 1))
                    vT_sb = work_pool.tile([P, TQ], F32, tag="vT")
                    nc.scalar.activation(vT_sb[:], mmv[:], AF.Identity, bias=bv_sb[:])
                    for j in range(TQ // P):
                        ptr = psum_small.tile([P, P], F32, tag="small")
                        nc.tensor.transpose(ptr[:], vT_sb[:, bass.ds(j * P, P)], ident[:])
                        kc_global = tt * (TQ // P) + j
                        nc.vector.tensor_copy(
                            vA[:, kc_global, :, 0:HD],
                            ptr[:].rearrange("p (h d) -> p h d", h=HPC),
                        )

                # ---- attention + output projection for batch b ----
                for qc in range(TPB):
                    S = bass.ds(qc * TQ, TQ)
                    otile = ot_pool.tile([P, TQ], F32R, tag="ot")
                    for h in range(HPC):
                        hs = bass.ds(h * HD, HD)
                        n_kc = 4 * (qc + 1) if causal else KCB
                        po = psum_o.tile([HD + 1, TQ], F32, tag="opsum")
                        for kc in range(n_kc):
                            ps = psum_s.tile([P, TQ], F32, tag="spsum")
                            nc.tensor.matmul(
                                ps[:],
                                kT[hs, bass.ds(kc * P, P)],
                                qT[hs, S],
                                start=True, stop=True)
                            et = exp_pool.tile([P, TQ], F32R, tag="exp")
                            nc.scalar.activation(et[:], ps[:], AF.Exp, scale=SCALE)
                            if causal and kc >= 4 * qc:
                                m = kc - 4 * qc
                                nc.vector.tensor_tensor(
                                    et[:], et[:], diag_sb[:, m, :], MULT)
                            nc.tensor.matmul(po[:], vA[:, kc, h, :], et[:],
                                             start=(kc == 0), stop=(kc == n_kc - 1))
                        # normalize: row HD holds the softmax denominators
                        recip = work_pool.tile([1, TQ], F32R, tag="recip")
                        with nc.allow_low_precision(reason="softmax recip bcast"):
                            nc.vector.reciprocal(recip[:], po[HD:HD + 1, :])
                        pb = psum_small.tile([HD, TQ], F32, tag="small")
                        nc.tensor.matmul(pb[:], ones_sb[:], recip[:],
                                         start=True, stop=True)
                        rb = work_pool.tile([HD, TQ], F32, tag="rb")
                        nc.any.tensor_copy(rb[:], pb[:])
                        nc.vector.tensor_tensor(otile[hs, :], po[0:HD, :], rb[:], MULT)
                    # output projection for these TQ tokens
                    for tk in range(TQ // P):
                        osb = out_pool.tile([P, D], F32, tag="osb")
                        for nh in range(D // TQ):
                            pop = psum_mm.tile([P, TQ], F32, tag="mm512")
                            nc.tensor.matmul(
                                pop[:],
                                otile[:, bass.ds(tk * P, P)],
                                wo_sb[:, bass.ds(nh * TQ, TQ)],
                                start=True, stop=True)
                            nc.any.tensor_copy(
                                osb[:, bass.ds(nh * TQ, TQ)], pop[:])
                        row0 = b * T + qc * TQ + tk * P
                        nc.sync.dma_start(out[bass.ds(row0, P), :], osb[:])

    nc.compile()
    return nc


def _get_nc(causal: bool, repeat: int = 1):
    key = (causal, repeat)
    if key not in _cache:
        _cache[key] = build_nc(causal, repeat)
    return _cache[key]


def _host_fallback(x, mask, cos, sin, Wq, bq, Wk, bk, Wv, bv, Wo, bo):
    """Pure-numpy reference path for arbitrary masks (never hit in practice)."""
    def rotate_half(a):
        return np.concatenate((-a[..., a.shape[-1] // 2:],
                               a[..., :a.shape[-1] // 2]), axis=-1)
    q = (x @ Wq + bq).reshape(B, T, H, HD).transpose(0, 2, 1, 3)
    k = (x @ Wk + bk).reshape(B, T, H, HD).transpose(0, 2, 1, 3)
    v = (x @ Wv + bv).reshape(B, T, H, HD).transpose(0, 2, 1, 3)
    q = q * cos + rotate_half(q) * sin
    k = k * cos + rotate_half(k) * sin
    outp = np.empty((B, H, T, HD), np.float32)
    for bi in range(B):
        for hi in range(H):
            s = (q[bi, hi] @ k[bi, hi].T) * SCALE
            s = np.where(mask[0, 0], s, -np.inf)
            s = s - s.max(-1, keepdims=True)
            e = np.exp(s)
            p = e / e.sum(-1, keepdims=True)
            outp[bi, hi] = p @ v[bi, hi]
    o = outp.transpose(0, 2, 1, 3).reshape(B, T, D)
    return (o @ Wo + bo).astype(np.float32)


def kernel(x, mask, cos, sin, Wq, bq, Wk, bk, Wv, bv, Wo, bo, _want_results=False, _trace=False):
    x = np.asarray(x); mask = np.asarray(mask)
    cos = np.asarray(cos); sin = np.asarray(sin)
    Wq = np.asarray(Wq, np.float32); Wk = np.asarray(Wk, np.float32)
    Wv = np.asarray(Wv, np.float32); Wo = np.asarray(Wo, np.float32)
    bq = np.asarray(bq, np.float32); bk = np.asarray(bk, np.float32)
    bv = np.asarray(bv, np.float32); bo = np.asarray(bo, np.float32)

    m2 = np.asarray(mask).reshape(T, T)
    tril = np.tril(np.ones((T, T), dtype=bool))
    if np.array_equal(m2, tril):
        causal = True
    elif m2.all():
        causal = False
    else:
        return _host_fallback(x, mask, cos, sin, Wq, bq, Wk, bk, Wv, bv, Wo, bo)

    # ---- host-side prep ----
    X2 = np.ascontiguousarray(x.reshape(NT, D).astype(np.float32))
    xt = round_fp32r(
        np.ascontiguousarray(X2.T.reshape(KD, P, NT).transpose(1, 0, 2)))
    cosT = np.ascontiguousarray(cos.reshape(T, HD).T.astype(np.float32))
    sinT = np.ascontiguousarray(sin.reshape(T, HD).T.astype(np.float32))
    cosd = np.ascontiguousarray(np.tile(cosT, (HPC, 1)))
    rowsign = np.where((np.arange(DPC) % HD) < (HD // 2), -1.0, 1.0).astype(np.float32)
    sind = np.ascontiguousarray(np.tile(sinT, (HPC, 1)) * rowsign[:, None])
    # diagonal-block masks: allow key (128m+p) <= q col c
    pp = np.arange(P)[:, None]
    cc = np.arange(TQ)[None, :]
    diagm = np.stack([(P * m + pp <= cc) for m in range(4)], axis=1)
    diagm = np.ascontiguousarray(diagm.astype(np.float32))

    in_maps = []
    for c in range(NCORES):
        cs = slice(c * DPC, (c + 1) * DPC)
        wq3 = round_fp32r(np.ascontiguousarray(
            Wq[:, cs].reshape(KD, P, DPC).transpose(1, 0, 2)))
        wk3 = round_fp32r(np.ascontiguousarray(
            Wk[:, cs].reshape(KD, P, DPC).transpose(1, 0, 2)))
        wv3 = round_fp32r(np.ascontiguousarray(
            Wv[:, cs].reshape(KD, P, DPC).transpose(1, 0, 2)))
        wo2 = round_fp32r(np.ascontiguousarray(Wo[cs, :]))
        in_maps.append(dict(
            xt=xt, wq=wq3, wk=wk3, wv=wv3, wo=wo2,
            bq=np.ascontiguousarray(bq[cs])[:, None],
            bk=np.ascontiguousarray(bk[cs])[:, None],
            bv=np.ascontiguousarray(bv[cs])[:, None],
            cosd=cosd, sind=sind, diag=diagm,
        ))

    global _last_in_maps
    _last_in_maps = in_maps
    nc = _get_nc(causal)
    res = run_bass_kernel_spmd(nc, in_maps, list(range(NCORES)), trace=_trace)
    acc = np.zeros((NT, D), np.float64)
    for r in res.results:
        acc += r["out"]
    final = (acc + bo).astype(np.float32).reshape(B, T, D)
    if _want_results:
        return final, res
    return final


# revision 15
# speedup vs baseline: 3.9086x; 3.9086x over previous
"""Trainium2 Bass kernel for causal multi-head attention with RoPE.

Problem: B=4, T=2048, D=1024, H=16 heads (HD=64), fp32.
  q,k,v = x@W* + b*;  RoPE(q,k);  causal softmax(q k^T / 8) @ v;  out @ Wo + bo.

Sharding: tensor-parallel over heads — each of the 8 cores owns 2 heads
(a 128-column slice of Wq/Wk/Wv and 128-row slice of Wo) and computes a
partial [B*T, D] output; the host sums the 8 partials and adds bo.

Device layout (per core, everything transpose-free):
  - host passes x pre-transposed as xT [128, 8, B*T] (D on partitions,
    8 K-chunks along free dim) pre-rounded to fp32r
  - qT,kT [128(=2 heads x 64d), T] per batch via matmul(lhsT=W chunk,
    rhs=xT chunk); RoPE applied in [d,t] layout with host cosT/sinT
  - scores TRANSPOSED: S^T[key,q] = matmul(lhsT=kT head, rhs=qT head);
    softmax runs over the partition dim with no max-subtraction (scores
    here are bounded ~|s|<3), sums taken by a ones-column appended to V
  - V in natural [key, d] layout via PE transpose of vT
  - O^T accumulates in PSUM via matmul(lhsT=V_aug, rhs=exp(S^T)); row 64
    is the softmax denominator; normalize via reciprocal + ones-broadcast
  - output projection: matmul(lhsT=O^T chunk, rhs=Wo rows) -> partial out

fp32r (fp32 with 11-bit mantissa, 4x PE throughput) is used for all
matmul operands; accumulation stays fp32 in PSUM.
"""
import numpy as np
import sys

sys.path.insert(0, "/opt/trn_rl_repo")

import concourse.bass as bass
import concourse.mybir as mybir
import concourse.tile as tile
from concourse import bacc
from concourse.bass_utils import run_bass_kernel_spmd
from concourse.masks import make_identity

B, T, D, H = 4, 2048, 1024, 16
HD = D // H            # 64
NT = B * T             # 8192 tokens
NCORES = 8
HPC = H // NCORES      # 2 heads per core
DPC = HPC * HD         # 128 d per core
P = 128
KD = D // P            # 8 contraction chunks for the projections
TQ = 512               # q-chunk width (psum free dim)
NTT = NT // TQ         # 16 token tiles
TPB = T // TQ          # 4 q-chunks per batch
KCB = T // P           # 16 key chunks per batch
SCALE = 1.0 / np.sqrt(HD)

F32 = mybir.dt.float32
F32R = mybir.dt.float32r
AF = mybir.ActivationFunctionType
MULT = mybir.AluOpType.mult

_cache = {}
_last_in_maps = None


def round_fp32r(a: np.ndarray) -> np.ndarray:
    """Round-to-nearest-even to fp32 with 11-bit mantissa (fp32r storage)."""
    b = np.ascontiguousarray(a, dtype=np.float32).view(np.uint32)
    r = (b + 0x7FF + ((b >> 12) & 1)) & np.uint32(0xFFFFF000)
    return r.view(np.float32)


def build_nc(causal: bool, repeat: int = 1, bias_zero: bool = True):
    nc = bacc.Bacc("TRN2", target_bir_lowering=False, debug=False,
                   enable_asserts=True, num_devices=NCORES)
    xt = nc.dram_tensor("xt", [P, KD, NT], F32R, kind="ExternalInput")
    wq = nc.dram_tensor("wq", [P, KD, DPC], F32R, kind="ExternalInput")
    wk = nc.dram_tensor("wk", [P, KD, DPC], F32R, kind="ExternalInput")
    wv = nc.dram_tensor("wv", [P, KD, DPC], F32R, kind="ExternalInput")
    wo = nc.dram_tensor("wo", [DPC, D], F32R, kind="ExternalInput")
    bq = nc.dram_tensor("bq", [DPC, 1], F32, kind="ExternalInput")
    bk = nc.dram_tensor("bk", [DPC, 1], F32, kind="ExternalInput")
    bv = nc.dram_tensor("bv", [DPC, 1], F32, kind="ExternalInput")
    cosd = nc.dram_tensor("cosd", [DPC, T], F32, kind="ExternalInput")
    sind = nc.dram_tensor("sind", [DPC, T], F32, kind="ExternalInput")
    diag = nc.dram_tensor("diag", [P, 4, TQ], F32, kind="ExternalInput")
    out = nc.dram_tensor("out", [NT, D], F32, kind="ExternalOutput")

    with tile.TileContext(nc) as tc:
        with (
            tc.tile_pool(name="const", bufs=1) as const_pool,
            tc.tile_pool(name="xs", bufs=3) as x_pool,
            tc.tile_pool(name="qk", bufs=2) as qk_pool,
            tc.tile_pool(name="work", bufs=2) as work_pool,
            tc.tile_pool(name="expp", bufs=4) as exp_pool,
            tc.tile_pool(name="ot", bufs=2) as ot_pool,
            tc.tile_pool(name="outp", bufs=3) as out_pool,
            tc.tile_pool(name="mm512", bufs=2, space="PSUM") as psum_mm,
            tc.tile_pool(name="spsum", bufs=2, space="PSUM") as psum_s,
            tc.tile_pool(name="opsum", bufs=2, space="PSUM") as psum_o,
        ):
            # resident constants
            wq_sb = const_pool.tile([P, KD, DPC], F32R)
            wk_sb = const_pool.tile([P, KD, DPC], F32R)
            wv_sb = const_pool.tile([P, KD, DPC], F32R)
            wo_sb = const_pool.tile([DPC, D], F32R)
            bq_sb = const_pool.tile([DPC, 1], F32)
            bk_sb = const_pool.tile([DPC, 1], F32)
            bv_sb = const_pool.tile([DPC, 1], F32)
            cos_sb = const_pool.tile([DPC, T], F32)
            sin_sb = const_pool.tile([DPC, T], F32)
            ident = const_pool.tile([P, P], F32)
            ones_sb = const_pool.tile([1, HD], F32R)
            onesf_sb = const_pool.tile([1, HD], F32)
            nc.sync.dma_start(wq_sb[:], wq[:])
            nc.sync.dma_start(wk_sb[:], wk[:])
            nc.sync.dma_start(wv_sb[:], wv[:])
            nc.sync.dma_start(wo_sb[:], wo[:])
            nc.sync.dma_start(bq_sb[:], bq[:])
            nc.sync.dma_start(bk_sb[:], bk[:])
            nc.sync.dma_start(bv_sb[:], bv[:])
            nc.sync.dma_start(cos_sb[:], cosd[:])
            nc.sync.dma_start(sin_sb[:], sind[:])
            make_identity(nc, ident[:])
            nc.vector.memset(onesf_sb[:], 1.0)
            nc.vector.tensor_copy(ones_sb[:], onesf_sb[:])
            if causal:
                diag_sb = const_pool.tile([P, 4, TQ], F32)
                nc.sync.dma_start(diag_sb[:], diag[:])

            onescol_f = const_pool.tile([P, KCB, HPC, 1], F32)
            nc.vector.memset(onescol_f[:], 1.0)

            def rope(dst, src_psum, bias_sb, tt):
                """psum [128, TQ] -> dst slice with bias + RoPE, in [d,t] layout.
                sin_sb holds SIGNED sin (rows p%64<32 negated), so rotate-half
                is 4 plain partition-shifted copies (1-input ops may shift)."""
                S = bass.ds(tt * TQ, TQ)
                raw = work_pool.tile([P, TQ], F32, tag="rope_raw")
                if bias_zero:
                    nc.scalar.activation(raw[:], src_psum[:], AF.Copy)
                else:
                    nc.scalar.activation(raw[:], src_psum[:], AF.Identity,
                                         bias=bias_sb[:])
                rot = work_pool.tile([P, TQ], F32, tag="rope_rot")
                h2 = HD // 2
                for h in range(HPC):
                    o = h * HD
                    nc.vector.tensor_copy(rot[o:o + h2, :], raw[o + h2:o + HD, :])
                    nc.vector.tensor_copy(rot[o + h2:o + HD, :], raw[o:o + h2, :])
                nc.vector.tensor_tensor(rot[:], rot[:], sin_sb[:, S], MULT)
                nc.vector.tensor_tensor(dst[:, S], raw[:], cos_sb[:, S], MULT)
                nc.vector.tensor_add(dst[:, S], dst[:, S], rot[:])

            for rep in range(repeat):
              for b in range(B):
                # ---- projections for batch b: qT, kT, vA ----
                qT = qk_pool.tile([P, T], F32R, tag="qT")
                kT = qk_pool.tile([P, T], F32R, tag="kT")
                vA = qk_pool.tile([P, KCB, HPC, HD + 1], F32R, tag="vA")
                nc.vector.tensor_copy(vA[:, :, :, HD:HD + 1], onescol_f[:])
                for tt in range(TPB):
                    gt = b * TPB + tt  # global token tile
                    xt_sb = x_pool.tile([P, KD, TQ], F32R, tag="xt")
                    nc.sync.dma_start(xt_sb[:], xt[:, :, bass.ds(gt * TQ, TQ)])
                    for w_sb, b_sb, dstT in ((wq_sb, bq_sb, qT), (wk_sb, bk_sb, kT)):
                        mm = psum_mm.tile([P, TQ], F32, tag="mm512")
                        for kc in range(KD):
                            nc.tensor.matmul(mm[:], w_sb[:, kc, :], xt_sb[:, kc, :],
                                             start=(kc == 0), stop=(kc == KD - 1))
                        rope(dstT, mm, b_sb, tt)
                    # v: compute vT then PE-transpose into natural layout
                    mmv = psum_mm.tile([P, TQ], F32, tag="mm512")
                    for kc in range(KD):
                        nc.tensor.matmul(mmv[:], wv_sb[:, kc, :], xt_sb[:, kc, :],
                                         start=(kc == 0), stop=(kc == KD - 1))
                    vT_sb = work_pool.tile([P, TQ], F32, tag="vT")
                    if bias_zero:
                        nc.scalar.activation(vT_sb[:], mmv[:], AF.Copy)
                    else:
                        nc.scalar.activation(vT_sb[:], mmv[:], AF.Identity,
                                             bias=bv_sb[:])
                    for j in range(TQ // P):
                        ptr = psum_mm.tile([P, P], F32, tag="mm512")
                        nc.tensor.transpose(ptr[:], vT_sb[:, bass.ds(j * P, P)], ident[:])
                        kc_global = tt * (TQ // P) + j
                        nc.vector.tensor_copy(
                            vA[:, kc_global, :, 0:HD],
                            ptr[:].rearrange("p (h d) -> p h d", h=HPC),
                        )

                # ---- attention + output projection for batch b ----
                for qc in range(TPB):
                    S = bass.ds(qc * TQ, TQ)
                    otile = ot_pool.tile([P, TQ], F32R, tag="ot")
                    for h in range(HPC):
                        hs = bass.ds(h * HD, HD)
                        n_kc = 4 * (qc + 1) if causal else KCB
                        n_full = 4 * qc if causal else KCB
                        po = psum_o.tile([HD + 1, TQ], F32, tag="opsum")
                        # full blocks, two key-chunks per psum/exp pass
                        for pr in range(n_full // 2):
                            ps2 = psum_s.tile([P, 2, TQ], F32, tag="spsum")
                            for j in range(2):
                                kc = 2 * pr + j
                                nc.tensor.matmul(
                                    ps2[:, j, :],
                                    kT[hs, bass.ds(kc * P, P)],
                                    qT[hs, S],
                                    start=True, stop=True)
                            et2 = exp_pool.tile([P, 2, TQ], F32R, tag="exp2")
                            nc.scalar.activation(et2[:], ps2[:], AF.Exp,
                                                 scale=SCALE)
                            for j in range(2):
                                kc = 2 * pr + j
                                nc.tensor.matmul(po[:], vA[:, kc, h, :],
                                                 et2[:, j, :],
                                                 start=(kc == 0),
                                                 stop=(kc == n_kc - 1))
                        # diagonal blocks: masked columns [0, 128m) are dead,
                        # slice them out of every stage
                        for kc in range(n_full, n_kc):
                            m = kc - 4 * qc
                            c0 = m * P
                            cw = TQ - c0
                            cs = bass.ds(c0, cw)
                            ps = psum_s.tile([P, 2, TQ], F32, tag="spsum")
                            nc.tensor.matmul(
                                ps[:, 0, cs],
                                kT[hs, bass.ds(kc * P, P)],
                                qT[hs, bass.ds(qc * TQ + c0, cw)],
                                start=True, stop=True)
                            et = exp_pool.tile([P, TQ], F32R, tag="exp")
                            nc.scalar.activation(et[:, cs], ps[:, 0, cs], AF.Exp,
                                                 scale=SCALE)
                            nc.vector.tensor_tensor(
                                et[:, cs], et[:, cs], diag_sb[:, m, cs], MULT)
                            nc.tensor.matmul(po[:, cs], vA[:, kc, h, :], et[:, cs],
                                             start=(kc == 0), stop=(kc == n_kc - 1))
                        # normalize: row HD holds the softmax denominators
                        recip = work_pool.tile([1, TQ], F32R, tag="recip")
                        with nc.allow_low_precision(reason="softmax recip bcast"):
                            nc.vector.reciprocal(recip[:], po[HD:HD + 1, :])
                        pb = psum_mm.tile([HD, TQ], F32, tag="mm512")
                        nc.tensor.matmul(pb[:], ones_sb[:], recip[:],
                                         start=True, stop=True)
                        rb = work_pool.tile([HD, TQ], F32, tag="rb")
                        nc.any.tensor_copy(rb[:], pb[:])
                        nc.vector.tensor_tensor(otile[hs, :], po[0:HD, :], rb[:], MULT)
                    # output projection for these TQ tokens
                    for tk in range(TQ // P):
                        osb = out_pool.tile([P, D], F32, tag="osb")
                        for nh in range(D // TQ):
                            pop = psum_mm.tile([P, TQ], F32, tag="mm512")
                            nc.tensor.matmul(
                                pop[:],
                                otile[:, bass.ds(tk * P, P)],
                                wo_sb[:, bass.ds(nh * TQ, TQ)],
                                start=True, stop=True)
                            nc.any.tensor_copy(
                                osb[:, bass.ds(nh * TQ, TQ)], pop[:])
                        row0 = b * T + qc * TQ + tk * P
                        nc.sync.dma_start(out[bass.ds(row0, P), :], osb[:])

    nc.compile()
    return nc


def _get_nc(causal: bool, repeat: int = 1, bias_zero: bool = True):
    key = (causal, repeat, bias_zero)
    if key not in _cache:
        _cache[key] = build_nc(causal, repeat, bias_zero)
    return _cache[key]


def _host_fallback(x, mask, cos, sin, Wq, bq, Wk, bk, Wv, bv, Wo, bo):
    """Pure-numpy reference path for arbitrary masks (never hit in practice)."""
    def rotate_half(a):
        return np.concatenate((-a[..., a.shape[-1] // 2:],
                               a[..., :a.shape[-1] // 2]), axis=-1)
    q = (x @ Wq + bq).reshape(B, T, H, HD).transpose(0, 2, 1, 3)
    k = (x @ Wk + bk).reshape(B, T, H, HD).transpose(0, 2, 1, 3)
    v = (x @ Wv + bv).reshape(B, T, H, HD).transpose(0, 2, 1, 3)
    q = q * cos + rotate_half(q) * sin
    k = k * cos + rotate_half(k) * sin
    outp = np.empty((B, H, T, HD), np.float32)
    for bi in range(B):
        for hi in range(H):
            s = (q[bi, hi] @ k[bi, hi].T) * SCALE
            s = np.where(mask[0, 0], s, -np.inf)
            s = s - s.max(-1, keepdims=True)
            e = np.exp(s)
            p = e / e.sum(-1, keepdims=True)
            outp[bi, hi] = p @ v[bi, hi]
    o = outp.transpose(0, 2, 1, 3).reshape(B, T, D)
    return (o @ Wo + bo).astype(np.float32)


def kernel(x, mask, cos, sin, Wq, bq, Wk, bk, Wv, bv, Wo, bo, _want_results=False, _trace=False):
    x = np.asarray(x); mask = np.asarray(mask)
    cos = np.asarray(cos); sin = np.asarray(sin)
    Wq = np.asarray(Wq, np.float32); Wk = np.asarray(Wk, np.float32)
    Wv = np.asarray(Wv, np.float32); Wo = np.asarray(Wo, np.float32)
    bq = np.asarray(bq, np.float32); bk = np.asarray(bk, np.float32)
    bv = np.asarray(bv, np.float32); bo = np.asarray(bo, np.float32)

    m2 = np.asarray(mask).reshape(T, T)
    tril = np.tril(np.ones((T, T), dtype=bool))
    if np.array_equal(m2, tril):
        causal = True
    elif m2.all():
        causal = False
    else:
        return _host_fallback(x, mask, cos, sin, Wq, bq, Wk, bk, Wv, bv, Wo, bo)

    # ---- host-side prep ----
    X2 = np.ascontiguousarray(x.reshape(NT, D).astype(np.float32))
    xt = round_fp32r(
        np.ascontiguousarray(X2.T.reshape(KD, P, NT).transpose(1, 0, 2)))
    cosT = np.ascontiguousarray(cos.reshape(T, HD).T.astype(np.float32))
    sinT = np.ascontiguousarray(sin.reshape(T, HD).T.astype(np.float32))
    cosd = np.ascontiguousarray(np.tile(cosT, (HPC, 1)))
    rowsign = np.where((np.arange(DPC) % HD) < (HD // 2), -1.0, 1.0).astype(np.float32)
    sind = np.ascontiguousarray(np.tile(sinT, (HPC, 1)) * rowsign[:, None])
    # diagonal-block masks: allow key (128m+p) <= q col c
    pp = np.arange(P)[:, None]
    cc = np.arange(TQ)[None, :]
    diagm = np.stack([(P * m + pp <= cc) for m in range(4)], axis=1)
    diagm = np.ascontiguousarray(diagm.astype(np.float32))

    in_maps = []
    for c in range(NCORES):
        cs = slice(c * DPC, (c + 1) * DPC)
        wq3 = round_fp32r(np.ascontiguousarray(
            Wq[:, cs].reshape(KD, P, DPC).transpose(1, 0, 2)))
        wk3 = round_fp32r(np.ascontiguousarray(
            Wk[:, cs].reshape(KD, P, DPC).transpose(1, 0, 2)))
        wv3 = round_fp32r(np.ascontiguousarray(
            Wv[:, cs].reshape(KD, P, DPC).transpose(1, 0, 2)))
        wo2 = round_fp32r(np.ascontiguousarray(Wo[cs, :]))
        in_maps.append(dict(
            xt=xt, wq=wq3, wk=wk3, wv=wv3, wo=wo2,
            bq=np.ascontiguousarray(bq[cs])[:, None],
            bk=np.ascontiguousarray(bk[cs])[:, None],
            bv=np.ascontiguousarray(bv[cs])[:, None],
            cosd=cosd, sind=sind, diag=diagm,
        ))

    global _last_in_maps
    _last_in_maps = in_maps
    bias_zero = not (bq.any() or bk.any() or bv.any())
    nc = _get_nc(causal, 1, bias_zero)
    res = run_bass_kernel_spmd(nc, in_maps, list(range(NCORES)), trace=_trace)
    acc = np.zeros((NT, D), np.float64)
    for r in res.results:
        acc += r["out"]
    final = (acc + bo).astype(np.float32).reshape(B, T, D)
    if _want_results:
        return final, res
    return final


# revision 16
# speedup vs baseline: 4.6735x; 1.1957x over previous
"""Trainium2 Bass kernel for causal multi-head attention with RoPE.

Problem: B=4, T=2048, D=1024, H=16 heads (HD=64), fp32.
  q,k,v = x@W* + b*;  RoPE(q,k);  causal softmax(q k^T / 8) @ v;  out @ Wo + bo.

Sharding: tensor-parallel over heads — each of the 8 cores owns 2 heads
(a 128-column slice of Wq/Wk/Wv and 128-row slice of Wo) and computes a
partial [B*T, D] output; the host sums the 8 partials and adds bo.

Device layout (per core, everything transpose-free):
  - host passes x pre-transposed as xT [128, 8, B*T] (D on partitions,
    8 K-chunks along free dim) pre-rounded to fp32r
  - qT,kT [128(=2 heads x 64d), T] per batch via matmul(lhsT=W chunk,
    rhs=xT chunk); RoPE applied in [d,t] layout with host cosT/sinT
  - scores TRANSPOSED: S^T[key,q] = matmul(lhsT=kT head, rhs=qT head);
    softmax runs over the partition dim with no max-subtraction (scores
    here are bounded ~|s|<3), sums taken by a ones-column appended to V
  - V in natural [key, d] layout via PE transpose of vT
  - O^T accumulates in PSUM via matmul(lhsT=V_aug, rhs=exp(S^T)); row 64
    is the softmax denominator; normalize via reciprocal + ones-broadcast
  - output projection: matmul(lhsT=O^T chunk, rhs=Wo rows) -> partial out

fp32r (fp32 with 11-bit mantissa, 4x PE throughput) is used for all
matmul operands; accumulation stays fp32 in PSUM.
"""
import numpy as np
import sys

sys.path.insert(0, "/opt/trn_rl_repo")

import concourse.bass as bass
import concourse.mybir as mybir
import concourse.tile as tile
from concourse import bacc
from concourse.bass_utils import run_bass_kernel_spmd
from concourse.masks import make_identity

B, T, D, H = 4, 2048, 1024, 16
HD = D // H            # 64
NT = B * T             # 8192 tokens
NCORES = 8
HPC = H // NCORES      # 2 heads per core
DPC = HPC * HD         # 128 d per core
P = 128
KD = D // P            # 8 contraction chunks for the projections
TQ = 512               # q-chunk width (psum free dim)
NTT = NT // TQ         # 16 token tiles
TPB = T // TQ          # 4 q-chunks per batch
KCB = T // P           # 16 key chunks per batch
SCALE = 1.0 / np.sqrt(HD)

F32 = mybir.dt.float32
F32R = mybir.dt.float32r
AF = mybir.ActivationFunctionType
MULT = mybir.AluOpType.mult

_cache = {}
_last_in_maps = None


def round_fp32r(a: np.ndarray) -> np.ndarray:
    """Round-to-nearest-even to fp32 with 11-bit mantissa (fp32r storage)."""
    b = np.ascontiguousarray(a, dtype=np.float32).view(np.uint32)
    r = (b + 0x7FF + ((b >> 12) & 1)) & np.uint32(0xFFFFF000)
    return r.view(np.float32)


def build_nc(causal: bool, repeat: int = 1, bias_zero: bool = True):
    nc = bacc.Bacc("TRN2", target_bir_lowering=False, debug=False,
                   enable_asserts=True, num_devices=NCORES)
    xt = nc.dram_tensor("xt", [P, KD, NT], F32R, kind="ExternalInput")
    wq = nc.dram_tensor("wq", [P, KD, DPC], F32R, kind="ExternalInput")
    wk = nc.dram_tensor("wk", [P, KD, DPC], F32R, kind="ExternalInput")
    wv = nc.dram_tensor("wv", [P, KD, DPC], F32R, kind="ExternalInput")
    wo = nc.dram_tensor("wo", [DPC, D], F32R, kind="ExternalInput")
    bq = nc.dram_tensor("bq", [DPC, 1], F32, kind="ExternalInput")
    bk = nc.dram_tensor("bk", [DPC, 1], F32, kind="ExternalInput")
    bv = nc.dram_tensor("bv", [DPC, 1], F32, kind="ExternalInput")
    cosd = nc.dram_tensor("cosd", [DPC, T], F32, kind="ExternalInput")
    sind = nc.dram_tensor("sind", [DPC, T], F32, kind="ExternalInput")
    diag = nc.dram_tensor("diag", [P, 4, TQ], F32, kind="ExternalInput")
    out = nc.dram_tensor("out", [NT, D], F32, kind="ExternalOutput")

    with tile.TileContext(nc) as tc:
        with (
            tc.tile_pool(name="const", bufs=1) as const_pool,
            tc.tile_pool(name="xs", bufs=3) as x_pool,
            tc.tile_pool(name="qk", bufs=2) as qk_pool,
            tc.tile_pool(name="work", bufs=2) as work_pool,
            tc.tile_pool(name="expp", bufs=4) as exp_pool,
            tc.tile_pool(name="ot", bufs=2) as ot_pool,
            tc.tile_pool(name="outp", bufs=3) as out_pool,
            tc.tile_pool(name="mm512", bufs=2, space="PSUM") as psum_mm,
            tc.tile_pool(name="spsum", bufs=2, space="PSUM") as psum_s,
            tc.tile_pool(name="opsum", bufs=2, space="PSUM") as psum_o,
        ):
            # resident constants
            wq_sb = const_pool.tile([P, KD, DPC], F32R)
            wk_sb = const_pool.tile([P, KD, DPC], F32R)
            wv_sb = const_pool.tile([P, KD, DPC], F32R)
            wo_sb = const_pool.tile([DPC, D], F32R)
            bq_sb = const_pool.tile([DPC, 1], F32)
            bk_sb = const_pool.tile([DPC, 1], F32)
            bv_sb = const_pool.tile([DPC, 1], F32)
            cos_sb = const_pool.tile([DPC, T], F32)
            sin_sb = const_pool.tile([DPC, T], F32)
            ident = const_pool.tile([P, P], F32)
            ones_sb = const_pool.tile([1, HD], F32R)
            onesf_sb = const_pool.tile([1, HD], F32)
            nc.sync.dma_start(wq_sb[:], wq[:])
            nc.sync.dma_start(wk_sb[:], wk[:])
            nc.sync.dma_start(wv_sb[:], wv[:])
            nc.sync.dma_start(wo_sb[:], wo[:])
            nc.sync.dma_start(bq_sb[:], bq[:])
            nc.sync.dma_start(bk_sb[:], bk[:])
            nc.sync.dma_start(bv_sb[:], bv[:])
            nc.sync.dma_start(cos_sb[:], cosd[:])
            nc.sync.dma_start(sin_sb[:], sind[:])
            make_identity(nc, ident[:])
            nc.vector.memset(onesf_sb[:], 1.0)
            nc.vector.tensor_copy(ones_sb[:], onesf_sb[:])
            if causal:
                diag_sb = const_pool.tile([P, 4, TQ], F32)
                nc.sync.dma_start(diag_sb[:], diag[:])

            onescol_f = const_pool.tile([P, KCB, HPC, 1], F32)
            nc.vector.memset(onescol_f[:], 1.0)

            def rope(dst, src_psum, bias_sb, tt):
                """psum [128, TQ] -> dst slice with bias + RoPE, in [d,t] layout.
                sin_sb holds SIGNED sin (rows p%64<32 negated), so rotate-half
                is 4 plain partition-shifted copies (1-input ops may shift)."""
                S = bass.ds(tt * TQ, TQ)
                raw = work_pool.tile([P, TQ], F32, tag="rope_raw")
                if bias_zero:
                    nc.scalar.activation(raw[:], src_psum[:], AF.Copy)
                else:
                    nc.scalar.activation(raw[:], src_psum[:], AF.Identity,
                                         bias=bias_sb[:])
                rot = work_pool.tile([P, TQ], F32, tag="rope_rot")
                h2 = HD // 2
                for h in range(HPC):
                    o = h * HD
                    nc.any.tensor_copy(rot[o:o + h2, :], raw[o + h2:o + HD, :])
                    nc.any.tensor_copy(rot[o + h2:o + HD, :], raw[o:o + h2, :])
                nc.vector.tensor_tensor(rot[:], rot[:], sin_sb[:, S], MULT)
                nc.vector.tensor_tensor(dst[:, S], raw[:], cos_sb[:, S], MULT)
                nc.vector.tensor_add(dst[:, S], dst[:, S], rot[:])

            for rep in range(repeat):
              for b in range(B):
                # ---- projections for batch b: qT, kT, vA ----
                qT = qk_pool.tile([P, T], F32R, tag="qT")
                kT = qk_pool.tile([P, T], F32R, tag="kT")
                vA = qk_pool.tile([P, KCB, HPC, HD + 1], F32R, tag="vA")
                nc.vector.tensor_copy(vA[:, :, :, HD:HD + 1], onescol_f[:])
                for tt in range(TPB):
                    gt = b * TPB + tt  # global token tile
                    xt_sb = x_pool.tile([P, KD, TQ], F32R, tag="xt")
                    nc.sync.dma_start(xt_sb[:], xt[:, :, bass.ds(gt * TQ, TQ)])
                    for w_sb, b_sb, dstT in ((wq_sb, bq_sb, qT), (wk_sb, bk_sb, kT)):
                        mm = psum_mm.tile([P, TQ], F32, tag="mm512")
                        for kc in range(KD):
                            nc.tensor.matmul(mm[:], w_sb[:, kc, :], xt_sb[:, kc, :],
                                             start=(kc == 0), stop=(kc == KD - 1))
                        rope(dstT, mm, b_sb, tt)
                    # v: compute vT then PE-transpose into natural layout
                    mmv = psum_mm.tile([P, TQ], F32, tag="mm512")
                    for kc in range(KD):
                        nc.tensor.matmul(mmv[:], wv_sb[:, kc, :], xt_sb[:, kc, :],
                                         start=(kc == 0), stop=(kc == KD - 1))
                    vT_sb = work_pool.tile([P, TQ], F32, tag="vT")
                    if bias_zero:
                        nc.scalar.activation(vT_sb[:], mmv[:], AF.Copy)
                    else:
                        nc.scalar.activation(vT_sb[:], mmv[:], AF.Identity,
                                             bias=bv_sb[:])
                    for j in range(TQ // P):
                        ptr = psum_mm.tile([P, P], F32, tag="mm512")
                        nc.tensor.transpose(ptr[:], vT_sb[:, bass.ds(j * P, P)], ident[:])
                        kc_global = tt * (TQ // P) + j
                        nc.vector.tensor_copy(
                            vA[:, kc_global, :, 0:HD],
                            ptr[:].rearrange("p (h d) -> p h d", h=HPC),
                        )

                # ---- attention + output projection for batch b ----
                for qc in range(TPB):
                    S = bass.ds(qc * TQ, TQ)
                    otile = ot_pool.tile([P, TQ], F32R, tag="ot")
                    for h in range(HPC):
                        hs = bass.ds(h * HD, HD)
                        n_kc = 4 * (qc + 1) if causal else KCB
                        n_full = 4 * qc if causal else KCB
                        po = psum_o.tile([HD + 1, TQ], F32, tag="opsum")
                        # full blocks, two key-chunks per psum/exp pass
                        for pr in range(n_full // 2):
                            ps2 = psum_s.tile([P, 2, TQ], F32, tag="spsum")
                            for j in range(2):
                                kc = 2 * pr + j
                                nc.tensor.matmul(
                                    ps2[:, j, :],
                                    kT[hs, bass.ds(kc * P, P)],
                                    qT[hs, S],
                                    start=True, stop=True)
                            et2 = exp_pool.tile([P, 2, TQ], F32R, tag="exp2")
                            nc.scalar.activation(et2[:], ps2[:], AF.Exp,
                                                 scale=SCALE)
                            for j in range(2):
                                kc = 2 * pr + j
                                nc.tensor.matmul(po[:], vA[:, kc, h, :],
                                                 et2[:, j, :],
                                                 start=(kc == 0),
                                                 stop=(kc == n_kc - 1))
                        # diagonal blocks: masked columns [0, 128m) are dead,
                        # slice them out of every stage
                        for kc in range(n_full, n_kc):
                            m = kc - 4 * qc
                            c0 = m * P
                            cw = TQ - c0
                            cs = bass.ds(c0, cw)
                            ps = psum_s.tile([P, 2, TQ], F32, tag="spsum")
                            nc.tensor.matmul(
                                ps[:, 0, cs],
                                kT[hs, bass.ds(kc * P, P)],
                                qT[hs, bass.ds(qc * TQ + c0, cw)],
                                start=True, stop=True)
                            et = exp_pool.tile([P, TQ], F32R, tag="exp")
                            nc.scalar.activation(et[:, cs], ps[:, 0, cs], AF.Exp,
                                                 scale=SCALE)
                            nc.vector.tensor_tensor(
                                et[:, cs], et[:, cs], diag_sb[:, m, cs], MULT)
                            nc.tensor.matmul(po[:, cs], vA[:, kc, h, :], et[:, cs],
                                             start=(kc == 0), stop=(kc == n_kc - 1))
                        # normalize: row HD holds the softmax denominators
                        recip = work_pool.tile([1, TQ], F32R, tag="recip")
                        with nc.allow_low_precision(reason="softmax recip bcast"):
                            nc.vector.reciprocal(recip[:], po[HD:HD + 1, :])
                        pb = psum_mm.tile([HD, TQ], F32, tag="mm512")
                        nc.tensor.matmul(pb[:], ones_sb[:], recip[:],
                                         start=True, stop=True)
                        rb = work_pool.tile([HD, TQ], F32, tag="rb")
                        nc.any.tensor_copy(rb[:], pb[:])
                        nc.vector.tensor_tensor(otile[hs, :], po[0:HD, :], rb[:], MULT)
                    # output projection for these TQ tokens
                    for tk in range(TQ // P):
                        osb = out_pool.tile([P, D], F32, tag="osb")
                        for nh in range(D // TQ):
                            pop = psum_mm.tile([P, TQ], F32, tag="mm512")
                            nc.tensor.matmul(
                                pop[:],
                                otile[:, bass.ds(tk * P, P)],
                                wo_sb[:, bass.ds(nh * TQ, TQ)],
                                start=True, stop=True)
                            nc.any.tensor_copy(
                                osb[:, bass.ds(nh * TQ, TQ)], pop[:])
                        row0 = b * T + qc * TQ + tk * P
                        nc.sync.dma_start(out[bass.ds(row0, P), :], osb[:])

    nc.compile()
    return nc


def _get_nc(causal: bool, repeat: int = 1, bias_zero: bool = True):
    key = (causal, repeat, bias_zero)
    if key not in _cache:
        _cache[key] = build_nc(causal, repeat, bias_zero)
    return _cache[key]


def _host_fallback(x, mask, cos, sin, Wq, bq, Wk, bk, Wv, bv, Wo, bo):
    """Pure-numpy reference path for arbitrary masks (never hit in practice)."""
    def rotate_half(a):
        return np.concatenate((-a[..., a.shape[-1] // 2:],
                               a[..., :a.shape[-1] // 2]), axis=-1)
    q = (x @ Wq + bq).reshape(B, T, H, HD).transpose(0, 2, 1, 3)
    k = (x @ Wk + bk).reshape(B, T, H, HD).transpose(0, 2, 1, 3)
    v = (x @ Wv + bv).reshape(B, T, H, HD).transpose(0, 2, 1, 3)
    q = q * cos + rotate_half(q) * sin
    k = k * cos + rotate_half(k) * sin
    outp = np.empty((B, H, T, HD), np.float32)
    for bi in range(B):
        for hi in range(H):
            s = (q[bi, hi] @ k[bi, hi].T) * SCALE
            s = np.where(mask[0, 0], s, -np.inf)
            s = s - s.max(-1, keepdims=True)
            e = np.exp(s)
            p = e / e.sum(-1, keepdims=True)
            outp[bi, hi] = p @ v[bi, hi]
    o = outp.transpose(0, 2, 1, 3).reshape(B, T, D)
    return (o @ Wo + bo).astype(np.float32)


def kernel(x, mask, cos, sin, Wq, bq, Wk, bk, Wv, bv, Wo, bo, _want_results=False, _trace=False):
    x = np.asarray(x); mask = np.asarray(mask)
    cos = np.asarray(cos); sin = np.asarray(sin)
    Wq = np.asarray(Wq, np.float32); Wk = np.asarray(Wk, np.float32)
    Wv = np.asarray(Wv, np.float32); Wo = np.asarray(Wo, np.float32)
    bq = np.asarray(bq, np.float32); bk = np.asarray(bk, np.float32)
    bv = np.asarray(bv, np.float32); bo = np.asarray(bo, np.float32)

    m2 = np.asarray(mask).reshape(T, T)
    tril = np.tril(np.ones((T, T), dtype=bool))
    if np.array_equal(m2, tril):
        causal = True
    elif m2.all():
        causal = False
    else:
        return _host_fallback(x, mask, cos, sin, Wq, bq, Wk, bk, Wv, bv, Wo, bo)

    # ---- host-side prep ----
    X2 = np.ascontiguousarray(x.reshape(NT, D).astype(np.float32))
    xt = round_fp32r(
        np.ascontiguousarray(X2.T.reshape(KD, P, NT).transpose(1, 0, 2)))
    cosT = np.ascontiguousarray(cos.reshape(T, HD).T.astype(np.float32))
    sinT = np.ascontiguousarray(sin.reshape(T, HD).T.astype(np.float32))
    cosd = np.ascontiguousarray(np.tile(cosT, (HPC, 1)))
    rowsign = np.where((np.arange(DPC) % HD) < (HD // 2), -1.0, 1.0).astype(np.float32)
    sind = np.ascontiguousarray(np.tile(sinT, (HPC, 1)) * rowsign[:, None])
    # diagonal-block masks: allow key (128m+p) <= q col c
    pp = np.arange(P)[:, None]
    cc = np.arange(TQ)[None, :]
    diagm = np.stack([(P * m + pp <= cc) for m in range(4)], axis=1)
    diagm = np.ascontiguousarray(diagm.astype(np.float32))

    in_maps = []
    for c in range(NCORES):
        cs = slice(c * DPC, (c + 1) * DPC)
        wq3 = round_fp32r(np.ascontiguousarray(
            Wq[:, cs].reshape(KD, P, DPC).transpose(1, 0, 2)))
        wk3 = round_fp32r(np.ascontiguousarray(
            Wk[:, cs].reshape(KD, P, DPC).transpose(1, 0, 2)))
        wv3 = round_fp32r(np.ascontiguousarray(
            Wv[:, cs].reshape(KD, P, DPC).transpose(1, 0, 2)))
        wo2 = round_fp32r(np.ascontiguousarray(Wo[cs, :]))
        in_maps.append(dict(
            xt=xt, wq=wq3, wk=wk3, wv=wv3, wo=wo2,
            bq=np.ascontiguousarray(bq[cs])[:, None],
            bk=np.ascontiguousarray(bk[cs])[:, None],
            bv=np.ascontiguousarray(bv[cs])[:, None],
            cosd=cosd, sind=sind, diag=diagm,
        ))

    global _last_in_maps
    _last_in_maps = in_maps
    bias_zero = not (bq.any() or bk.any() or bv.any())
    nc = _get_nc(causal, 1, bias_zero)
    res = run_bass_kernel_spmd(nc, in_maps, list(range(NCORES)), trace=_trace)
    acc = np.zeros((NT, D), np.float64)
    for r in res.results:
        acc += r["out"]
    final = (acc + bo).astype(np.float32).reshape(B, T, D)
    if _want_results:
        return final, res
    return final
